# revision 47
# baseline (speedup 1.0000x reference)
"""Trainium2 Bass kernel for nn_AFSLSTM (LayerNorm -> sigmoid feature gate ->
bidirectional 1-step LSTM -> tiny MLP head).

Sharding: data-parallel over the batch dim, 1024 rows per core, weights
replicated. No collectives; host concatenates per-core outputs.

Device-side math (per core, feature-major layout [feature_part, batch_free]):
the gate and LSTM matmuls (97% of PE work) run as fp8-e4m3 DoubleRow matmuls
(two 128-row contraction chunks per instruction, ~213ns/MM spacing covering
256xK at N=512), with weights pre-scaled by 64 on the host so the
uniform(-1/32,1/32) entries sit in fp8's normal range. The 1/64 descale
folds into the LN rsqrt (gate) and the activation `scale` operand (LSTM).
x ships twice (fp8 for the gate moving operand + LN stats, bf16 for the xg
product); xg is written as fp8 by the DVE so the LSTM matmuls consume it
directly. Head stays bf16.
  G0 = Q8(64*Wg*ln_g).Q8(x)                  (8 DoubleRow matmuls per j-chunk)
  gate = sigmoid((rsq/64)*(G0 - mu(x)c1q) + bg)    (c1q = colsums of the
        quantized weights; ALL groups park their PSUM to SBUF via ScalarE
        copies right at close -- parking decouples bank recycling from the
        DVE epilogue drain, which starts only after the rsq broadcast and
        would otherwise WAR-stall the first LSTM group ~4.7us on the tail
        banks -- and the j >= CLOSE_FROM=12 tail groups apply -mu*c1 as a
        K=1 bf16 close matmul BEFORE parking, so their drain flush is the
        cheap a_b-scale path instead of the scalar_tensor_tensor one)
  xg = fp8(x * gate)
  pre_{i,g,o} = Q8(64*W_ih[{i,g,o}]).xg      (f-gate dropped: c0 = 0)
  h = sigmoid(pre_o/64 + b_o) * tanh(sigmoid(pre_i/64 + b_i)*tanh(pre_g/64 + b_g))
  hid = relu(W1.feat + b1);  out = W2.hid + b2
LN statistics: BOTH batch sums (from fp8 x) and square sums (from host-shipped
fp8 x^2 pair-chunks) come from DoubleRow ones-matmuls interleaved into gate
j0's DMA-paced stream; mu -> -mu -> mu_b broadcast lands ~30us, the variance
chain (merged WS^2 scaling, mu^2 on ACT Square in parallel, Ln/Exp with
prefetched tables) broadcasts rsq ~42us; the post-loop drain then flushes all
16 parked groups in chunk order with each group's xg mul emitted one group
behind its sigmoid so the DVE FIFO never head-of-line blocks on ACT.
Schedule notes (trace-derived): engine preambles end ~5-7us; HAM warmup MMs
run off a GPSIMD-memset tile from ~5.5us; first gate MM ~13us paced by wg0 +
fp8-x pair arrival (DMA rings cost ~33ns/descriptor: 128-descriptor
transfers floor at ~4.4us); 4 LSTM weight groups prefetch on the sync queue
behind the bf16-x tail so the gate->LSTM handoff never starves; the fc-head
accumulation matmuls ride one group behind the LSTM feat writes into a
dedicated PSUM bank; the last LSTM group's epilogue -> head -> W2 -> out
chain runs per batch-half to pipeline the tail. Engine laws learned the hard
way: GPSIMD elementwise offload backfires (shared SBUF port pair with DVE);
sync and scalar share 8 one-transfer-at-a-time DMA rings and a busy-ring
trigger blocks its ENGINE queue, so bulk triggers never ride ACT; the tile
scheduler reorders per-engine programs, so critical cross-engine orderings
need explicit add_dep edges; single-partition DVE row ops run ~2.5ns/elem.
Measured: 274.2us baseline -> 266.9-268.0us (rel err 7.4e-3, gate 2e-2);
remaining: ~221us DoubleRow matmul floor, ~7us preamble, ~3us teardown,
~2.5us wave-1 ring pacing, ~2us drain/gate balance at the LSTM handoff.
"""

import numpy as np
import ml_dtypes

import concourse.bacc as bacc
import concourse.bass as bass
import concourse.mybir as mybir
import concourse.tile as tile
from concourse import bass_utils

BF16 = ml_dtypes.bfloat16
F8 = ml_dtypes.float8_e4m3  # TRN FP8_EXP4 (max finite 240)
NCORES = 8
B, F, H = 8192, 2048, 1024
BL = B // NCORES          # 1024 rows per core
KC = F // 128             # 16 contraction chunks
SC = KC // 2              # 8 DoubleRow super-chunks
NB = 2                    # batch sub-chunks per core
BW = BL // NB             # 512 (one PSUM bank of fp32)
NG_LSTM = 2 * 8 * 3       # dir x h-chunk x {i,f,g,o}->{i,g,o} weight groups
EPS = 1e-5
WS = 64.0                 # fp8 weight pre-scale
USE_CLOSE_MM = True       # rank-1 PE close matmul for the -mu*c1 term
NBL = 16                  # ALL gate groups park to SBUF via ACT copies right
                          # at close: the j12-15 banks otherwise wait for the
                          # end of the DVE drain and WAR-stall the first LSTM
                          # group ~4.7us. The j >= CLOSE_FROM groups get the
                          # rank-1 close matmul BEFORE parking, so their praw
                          # already holds -mu*c1 and their flush is the cheap
                          # a_b-scale path (the all-stt NBL=16 variant pushed
                          # the xg tail past the LSTM start and lost ~2us).
CLOSE_FROM = 12

_CACHE = {}


def _build_graph(has_c2=False):
    """has_c2: general path with a nonzero ln_b (rank-1 close matmuls on PE).
    The fast path (ln_b == 0) applies the -mu*c1 correction as a fused DVE
    scalar_tensor_tensor against a GPSIMD partition-broadcast of -mu, so PSUM
    groups close right at the last K-chunk and PE never waits on LN stats."""
    dt = mybir.dt
    AF = mybir.ActivationFunctionType
    OP = mybir.AluOpType
    DR = mybir.MatmulPerfMode.DoubleRow

    nc = bacc.Bacc("TRN2", target_bir_lowering=False, debug=False)

    xt_d = nc.dram_tensor("xt", (128, KC, BL), dt.bfloat16, kind="ExternalInput")
    xq_d = nc.dram_tensor("xq", (128, KC, BL), dt.float8e4, kind="ExternalInput")
    xs_d = nc.dram_tensor("xsq", (128, KC, BL), dt.float8e4, kind="ExternalInput")
    wg_d = nc.dram_tensor("wgm", (16, 128, KC, 128), dt.float8e4, kind="ExternalInput")
    wge_d = nc.dram_tensor("wge", (16, 2, 128), dt.bfloat16, kind="ExternalInput")
    wl_d = nc.dram_tensor("wlm", (16, 128, 3, KC, 128), dt.float8e4, kind="ExternalInput")
    bg_d = nc.dram_tensor("bgv", (128, 16), dt.float32, kind="ExternalInput")
    c1_d = nc.dram_tensor("c1v", (128, 16), dt.float32, kind="ExternalInput")
    bl_d = nc.dram_tensor("blv", (128, 48), dt.float32, kind="ExternalInput")
    w1_d = nc.dram_tensor("w1v", (128, KC * 64), dt.bfloat16, kind="ExternalInput")
    w2_d = nc.dram_tensor("w2v", (128, 1), dt.bfloat16, kind="ExternalInput")
    b1_d = nc.dram_tensor("b1v", (128, 1), dt.float32, kind="ExternalInput")
    b2_d = nc.dram_tensor("b2v", (1, 1), dt.float32, kind="ExternalInput")
    out_d = nc.dram_tensor("out", (1, BL), dt.float32, kind="ExternalOutput")

    with tile.TileContext(nc) as tc:
        with (
            tc.tile_pool(name="pers", bufs=1) as pers,
            tc.tile_pool(name="wpool", bufs=6) as wpool,
            tc.tile_pool(name="lpool", bufs=4) as lpool,
            tc.tile_pool(name="wepool", bufs=3) as wepool,
            tc.tile_pool(name="sqp", bufs=7) as sqp,
            tc.tile_pool(name="tmp", bufs=2) as tmp,
            tc.tile_pool(name="psum", bufs=7, space=bass.MemorySpace.PSUM) as psum,
            tc.tile_pool(name="hpp", bufs=1, space=bass.MemorySpace.PSUM) as hpp,
        ):
            # ---- persistent SBUF tensors ----
            xsb = pers.tile([128, KC * BL], dt.bfloat16)
            xq3 = pers.tile([128, KC, BL], dt.float8e4)
            xg3 = pers.tile([128, KC, BL], dt.float8e4)
            feat = pers.tile([128, KC * BL], dt.bfloat16)
            hid2 = pers.tile([128, BW], dt.bfloat16)
            a_b = pers.tile([128, BL], dt.bfloat16)      # rsq/WS broadcast tile
            mu_b = pers.tile([128, BL], dt.bfloat16)     # -mu broadcast tile
            ones128 = pers.tile([128, 1], dt.bfloat16)
            onesq = pers.tile([128, 2, 16], dt.float8e4)  # DR ones lhsT (col 0 used)
            wtile = pers.tile([128, 128], dt.bfloat16)   # HAM warmup operand
            # single-partition f32 rows (each costs free-bytes on every
            # partition, so reuse aggressively): s1row doubles as mu,
            # s2row as t1 -> ve -> lnv, orow doubles as the var rowtmp.
            s1row = pers.tile([1, BL], dt.float32)
            s2row = pers.tile([1, BL], dt.float32)
            rsqb = pers.tile([1, BL], dt.bfloat16)
            orow = pers.tile([1, BL], dt.float32)
            xe = pers.tile([2, BL], dt.bfloat16)         # rank-1 rhs rows (-mu, sqrt(ve))
            xe1s = (pers.tile([1, BL], dt.bfloat16)      # partition-0 staging, row 1
                    if has_c2 else None)
            bg_sb = pers.tile([128, 16], dt.float32)
            c1_sb = pers.tile([128, 16], dt.float32)
            bi_sb = pers.tile([128, 48], dt.float32)     # col = d*24 + part*8 + hm
            w1_sb = pers.tile([128, KC * 64], dt.bfloat16)
            w2_sb = pers.tile([128, 1], dt.bfloat16)
            b1_sb = pers.tile([128, 1], dt.float32)
            b2_sb = pers.tile([1, 1], dt.float32)

            nc.vector.memset(ones128[:], 1.0)
            nc.vector.memset(onesq[:], 1.0)
            nc.vector.memset(a_b[:], 0.0)
            # dummy matmuls on a zeroed tile keep the PE busy through the
            # wave-1 DMA wait so the HAM clock gate is at 8/8 (2.4GHz) when
            # the real gate stream starts. The operand is memset on GPSIMD,
            # whose ~5us engine preamble finishes ~2us before Vector's, so
            # the warmups begin right as the Tensor queue opens.
            nc.gpsimd.memset(wtile[:], 0.0)
            warmp = psum.tile([128, 128], dt.float32, tag="mm", name="warmp")
            for _ in range(60):
                nc.tensor.matmul(warmp[:], wtile[:], wtile[:],
                                 start=True, stop=True)

            # ---- input schedule. The sync and scalar engines share 8 DMA
            # rings; each ring runs ONE transfer at a time (~43GB/s) and a
            # trigger whose ring is busy blocks its ENGINE queue (so bulk
            # triggers must never ride a compute engine: the scalar/ACT
            # queue only gets the small wave-1 odd chunks, issued before any
            # ACT work exists). Deadline order on sync:
            #   wg0 (4 subs, first matmul ~9.5us) -> xq/xsq wave 1 (4.7MB,
            #   j0's DMA-paced stream consumes it as it lands) -> wg1..3 ->
            #   wg4..15 (chained 3-deep so they complete in j order) ->
            #   consts -> wets -> bf16 x (first gate epilogue needs xt0
            #   ~35us) -> wl0..3 LSTM prefetch -> in-loop wl4..15 (WAR-paced
            #   by lpool slots).
            # NOTE: never chain DMAs across streams (an xt<-wg edge deadlocks:
            # wg slots free only after gate MMs that sit behind the stats MMs
            # on the in-order PE queue, and stats need xt).
            from concourse.tile_rust import add_dep_helper
            wgt01 = [wpool.tile([128, KC, 128], dt.float8e4, tag="w", name=f"wg{j}")
                     for j in range(2)]

            for s in range(2):
                nc.sync.dma_start(
                    wgt01[0][:, s * 8:(s + 1) * 8, :],
                    wg_d[0, :, s * 8:(s + 1) * 8, :],
                )
            # wave 1: fp8 x and x^2 ship as PAIR-chunk transfers (2KB
            # contiguous runs per partition -- the ~33ns/descriptor ring
            # overhead dominates smaller runs), spread across both trigger
            # engines so all 8 rings fill in parallel
            xqd = [None] * SC
            xsqt = [None] * SC
            xsqd = [None] * SC

            def xsq_pair(sc, eng):
                t = sqp.tile([128, 2, BL], dt.float8e4, tag="xsq2", name=f"xsq{sc}")
                xsqt[sc] = t
                xsqd[sc] = eng.dma_start(t[:], xs_d[:, 2 * sc:2 * sc + 2, :])

            nc.sync.dma_start(xq3[:, 0, :], xq_d[:, 0, :])
            xqd[0] = nc.scalar.dma_start(xq3[:, 1, :], xq_d[:, 1, :])
            for sc in range(1, SC):
                eng = nc.sync if sc % 2 == 0 else nc.scalar
                xqd[sc] = eng.dma_start(xq3[:, 2 * sc:2 * sc + 2, :],
                                        xq_d[:, 2 * sc:2 * sc + 2, :])
            for sc in range(SC):
                xsq_pair(sc, nc.sync if sc % 2 == 1 else nc.scalar)
            xqdma = xqd[7]

            wgts = {0: wgt01[0], 1: wgt01[1]}
            wgd = {}
            for s in range(2):
                wgd[1] = nc.sync.dma_start(
                    wgt01[1][:, s * 8:(s + 1) * 8, :],
                    wg_d[1, :, s * 8:(s + 1) * 8, :],
                )
                add_dep_helper(wgd[1].ins, xqdma.ins, reason="wg1 behind wave 1")
            for j in range(2, 16):
                wgt = wpool.tile([128, KC, 128], dt.float8e4, tag="w", name=f"wg{j}")
                wd = nc.sync.dma_start(wgt[:], wg_d[j])
                # chain 3-deep so the weights complete in j order instead of
                # fair-sharing the rings and all landing together at the end
                dep = wgd[j - 3] if j - 3 >= 1 else None
                add_dep_helper(wd.ins, (dep or xqdma).ins,
                               reason="gate weight chain")
                wgd[j] = wd
                wgts[j] = wgt

            # ---- constants / small DMAs (held behind the gate weights) ----
            for sb_t, dr_t in [(bg_sb, bg_d), (c1_sb, c1_d), (bi_sb, bl_d),
                               (w1_sb, w1_d), (w2_sb, w2_d), (b1_sb, b1_d),
                               (b2_sb, b2_d)]:
                cd = nc.sync.dma_start(sb_t[:], dr_t[:, :])
                add_dep_helper(cd.ins, xqdma.ins, reason="const dma behind wave 1")

            # ---- LN statistics, entirely off the DVE/ACT hot paths: both
            # the batch sums s1 (from fp8 x) and the square sums s2 (from the
            # host-shipped fp8 x^2 pair-chunks) come from DoubleRow
            # ones-matmuls interleaved into gate j0's DMA-paced stream, so
            # they fill the PE idle slivers while wave 1 lands and the full
            # stats chain + broadcasts retire by ~34us (mu's fp8 quantization
            # error is ~6e-4 absolute, x^2's quantization adds ~0.2% to var
            # -- negligible vs the fp8 matmul noise).
            def emit_stat_mm(dst, rhs_pair, first, will_stop):
                for b in range(NB):
                    mm = nc.tensor.matmul(
                        dst[b][:], onesq[:, :, 0:1],
                        rhs_pair[:, :, b * BW:(b + 1) * BW],
                        start=first, stop=will_stop, perf_mode=DR,
                    )
                return mm

            def emit_mu():
                # evacuate s1, fold to mu = s1/F in place, and write the
                # -mu rank-1 row the (tail-group) close matmuls consume.
                for b in range(NB):
                    nc.vector.tensor_copy(s1row[:, b * BW:(b + 1) * BW], s1p[b][:])
                nc.vector.tensor_scalar_mul(s1row[:], s1row[:], 1.0 / F)
                nc.vector.tensor_scalar_mul(xe[0:1, :], s1row[:], -1.0)  # -> bf16
                nc.gpsimd.partition_broadcast(mu_b[:], xe[0:1, :])

            def emit_stats_chain():
                # variance chain from the s2 PSUM partials, then the
                # -mu / rsq partition-broadcasts on the idle GPSIMD.
                mu, lnv = s1row, s2row  # s1row already holds mu
                # WS^2-prescaled var chain (exp(-0.5 ln(.)) then yields
                # rsq/WS, undoing the fp8 weight pre-scale), run per batch
                # HALF: the drain's flush muls consume a_b per 512-col bank,
                # so the b0 half of the broadcast unblocks them ~3us before
                # the full-width chain would. mu^2 on ACT (Square, scale=WS)
                # overlaps the DVE ops; single-partition DVE is ~2.5ns/elem.
                for b in range(NB):
                    bs = slice(b * BW, (b + 1) * BW)
                    nc.vector.tensor_copy(s2row[:, bs], s2p[b][:])
                    nc.vector.tensor_scalar(s2row[:, bs], s2row[:, bs],
                                            WS * WS / F, WS * WS * EPS,
                                            OP.mult, OP.add)
                    sq = nc.scalar.activation(orow[:, bs], mu[:, bs],
                                              AF.Square, scale=WS)
                    if b == 0:
                        sq_ref.append(sq)
                    nc.vector.tensor_sub(s2row[:, bs], s2row[:, bs],
                                         orow[:, bs])  # WS^2(var+eps)
                    nc.scalar.activation(lnv[:, bs], s2row[:, bs], AF.Ln)
                    nc.scalar.activation(rsqb[:, bs], lnv[:, bs], AF.Exp,
                                         scale=-0.5)
                    nc.gpsimd.partition_broadcast(a_b[:, bs], rsqb[0:1, bs])
                if has_c2:
                    # engines can only write partition bases {0,32,64,96};
                    # stage row 1 on partition 0 and DMA it into place.
                    nc.scalar.activation(xe1s[:], lnv[:], AF.Exp, scale=0.5)
                    nc.sync.dma_start(xe[1:2, :], xe1s[:])

            def finish_gate(j, tps):
                gs = tmp.tile([128, BL], dt.bfloat16, tag="gs", bufs=3, name=f"gs{j}")
                nc.scalar.activation(gs[:], tps[:], AF.Sigmoid, bias=bg_sb[:, j:j + 1])
                return gs

            def finish_xg(j, gs):
                nc.vector.tensor_mul(xg3[:, j, :], xsb[:, j * BL:(j + 1) * BL], gs[:])

            def epi_backlog(j, prs, closed=False):
                # deferred path (stats weren't ready when the group closed):
                # praw was parked in SBUF; apply the -mu*c1 correction with a
                # DVE scalar_tensor_tensor, then scale, sigmoid, gate.
                tpre = tmp.tile([128, BL], dt.bfloat16, tag="tpre", bufs=3, name=f"tp{j}")
                for b in range(NB):
                    if has_c2 or closed:
                        nc.vector.tensor_mul(tpre[:, b * BW:(b + 1) * BW], prs[b][:],
                                             a_b[:, b * BW:(b + 1) * BW])
                    else:
                        nc.vector.scalar_tensor_tensor(
                            tpre[:, b * BW:(b + 1) * BW], mu_b[:, b * BW:(b + 1) * BW],
                            c1_sb[:, j:j + 1], prs[b][:], OP.mult, OP.add,
                        )
                if not (has_c2 or closed):
                    nc.vector.tensor_mul(tpre[:], tpre[:], a_b[:])
                return finish_gate(j, tpre)

            def epi_live(j, gp, closed):
                # live path: the rank-1 close matmul already applied -mu*c1 in
                # PSUM, so one DVE mul per bank both scales by rsq/WS and
                # evacuates (frees) the bank.
                tps = tmp.tile([128, BL], dt.bfloat16, tag="tps", name=f"ts{j}")
                for b in range(NB):
                    if closed:
                        nc.vector.tensor_mul(tps[:, b * BW:(b + 1) * BW], gp[b][:],
                                             a_b[:, b * BW:(b + 1) * BW])
                    else:
                        nc.vector.scalar_tensor_tensor(
                            tps[:, b * BW:(b + 1) * BW], mu_b[:, b * BW:(b + 1) * BW],
                            c1_sb[:, j:j + 1], gp[b][:], OP.mult, OP.add,
                        )
                if not closed:
                    nc.vector.tensor_mul(tps[:], tps[:], a_b[:])
                return finish_gate(j, tps)

            # ---- feature gate: 16 j-chunks x 8 DoubleRow super-chunks. PSUM
            # groups are evacuated to SBUF (bf16) right as they close; the
            # epilogues of j0..j3 are deferred until after the stats chain is
            # emitted so the DVE stream never blocks on the LN broadcasts.
            def gate_mm(gp, wgt, j, sc, will_close):
                out = []
                for b in range(NB):
                    out.append(nc.tensor.matmul(
                        gp[b][:], wgt[:, 2 * sc:2 * sc + 2, :],
                        xq3[:, 2 * sc:2 * sc + 2, b * BW:(b + 1) * BW],
                        start=(sc == 0), stop=(not will_close and sc == SC - 1),
                        perf_mode=DR,
                    ))
                return out

            def gate_close_evac(j, wet, gp, backlog):
                closed = (j >= CLOSE_FROM and USE_CLOSE_MM) or has_c2
                if (backlog is None and closed):
                    # rank-1 close matmul: += (-mu)(x)c1 (and sqrt(ve)(x)c2 in
                    # the general path) applied in PSUM at ~0.2us/bank of PE,
                    # saving the DVE scalar_tensor_tensor on the live path
                    K2 = 2 if has_c2 else 1
                    for b in range(NB):
                        nc.tensor.matmul(
                            gp[b][:], wet[0:K2, :], xe[0:K2, b * BW:(b + 1) * BW],
                            start=False, stop=True,
                        )
                prs = []
                for b in range(NB):
                    praw = tmp.tile([128, BW], dt.bfloat16, tag="praw", bufs=8,
                                    name=f"pr{j}_{b}")
                    # ScalarE copy: prompt bank release off the loaded DVE
                    cp = nc.scalar.copy(praw[:], gp[b][:])
                    if j == 1 and sq_ref:
                        add_dep_helper(cp.ins, sq_ref[0].ins,
                                       reason="square ahead of j1 parks")
                    prs.append(praw)
                backlog.append((j, prs, closed))

            wets = {}
            for j in range(16):
                if has_c2 or j >= CLOSE_FROM:
                    wet = wepool.tile([2, 128], dt.bfloat16, tag="we", bufs=6,
                                      name=f"we{j}")
                    wed = nc.sync.dma_start(wet[:], wge_d[j, :, :])
                    add_dep_helper(wed.ins, wgd[12].ins,
                                   reason="rank-1 rows late")
                    wets[j] = wet

            # bf16 x: consumed only by the gate epilogues (xg = x*gate), the
            # first of which waits for the stats broadcasts ~46us; stream it
            # behind the gate weights and consts on the sync queue.
            xtd = []
            for q in range(KC):
                xd = nc.sync.dma_start(xsb[:, q * BL:(q + 1) * BL], xt_d[:, q, :])
                add_dep_helper(xd.ins, wgd[min(15, 4 + q)].ins,
                               reason="bf16 x staggered behind weights")
                xtd.append(xd)

            # LSTM weight prefetch: the first 4 groups ride the sync queue
            # behind the bf16 x tail so the gate->LSTM handoff never starves
            # (they land ~50us, needed ~70); the rest stream in-loop,
            # WAR-paced by lpool slots. NEVER on the scalar engine: a
            # ring-busy trigger there blocks the whole ACT queue.
            wl_tiles = {}
            for dh in range(4):
                wlt = lpool.tile([128, 3, KC, 128], dt.float8e4, tag="wl",
                                 name=f"wl{dh}")
                wld = nc.sync.dma_start(wlt[:], wl_d[dh])
                add_dep_helper(wld.ins, xtd[12 + dh].ins,
                               reason="lstm prefetch behind bf16 x")
                wl_tiles[dh] = wlt

            # warm ONLY the Ln/Exp tables while ACT idles behind the wave-1
            # triggers: warming Sigmoid/Tanh too evicts these (the slots
            # cycle) and the real Ln/Exp reload on the rsq critical path
            for fn in (AF.Ln, AF.Exp):
                nc.scalar.activation(orow[0:1, 0:1], ones128[0:1, 0:1], fn)

            backlog = []
            live_defer = []
            sq_ref = []
            prev_last = None
            s1p = []
            s2p = []
            # Each j's first matmul is chained on the previous j's last: the
            # PE is serial anyway, but without the explicit edge the scheduler
            # interleaves the groups, closing them all late and starving PSUM
            # bank recycling (measured ~2-4us stall at j3/j4 on the bf16 ver).
            for j in range(16):
                wgt = wgts[j]
                wet = wets.get(j)
                gp = [psum.tile([128, BW], dt.float32, tag="mm", name=f"gp{j}_{b}")
                      for b in range(NB)]
                if j == 0:
                    s1p.extend(psum.tile([1, BW], dt.float32, tag="mm",
                                         name=f"s1p{b}") for b in range(NB))
                    s2p.extend(psum.tile([1, BW], dt.float32, tag="mm",
                                         name=f"s2p{b}") for b in range(NB))
                will_close = has_c2 or (j >= CLOSE_FROM and USE_CLOSE_MM)
                stat_last = None
                for sc in range(SC):
                    mms = gate_mm(gp, wgt, j, sc, will_close)
                    if sc == 0 and prev_last is not None:
                        add_dep_helper(mms[0].ins, prev_last.ins,
                                       reason="gate groups close in order")
                    if j == 0:
                        # the s1/s2 stat matmuls ride j0's DMA-paced
                        # chunk-pair stream, filling the PE idle slivers
                        # while wave 1 arrives (s2 lags 2 pairs so its
                        # fp8 x^2 transfer has landed)
                        stat_last = emit_stat_mm(
                            s1p, xq3[:, 2 * sc:2 * sc + 2, :],
                            sc == 0, sc == SC - 1)
                        if sc >= 2:
                            stat_last = emit_stat_mm(
                                s2p, xsqt[sc - 2][:], sc == 2, False)
                if j == 0:
                    for sc in (SC - 2, SC - 1):
                        stat_last = emit_stat_mm(s2p, xsqt[sc][:], False,
                                                 sc == SC - 1)
                prev_last = stat_last if j == 0 else mms[-1]
                gate_close_evac(j, wet, gp, backlog if j < NBL else None)
                if j == 0:
                    # stats resolve while j1/j2 stream: mu ~24us, broadcasts
                    # ~30-34us, so the j4+ epilogues run live
                    emit_mu()
                    emit_stats_chain()
            # post-loop DVE drain, in xg-production order: the 12 parked
            # flushes first (they only need mu_b/a_b, ~31us), then the tail
            # groups' PSUM evacuations. Each group's xg mul is emitted AFTER
            # the next group's stt/scale ops: the xg waits its sigmoid (ACT),
            # and emitting it inline would head-of-line block the DVE FIFO
            # for ~1.2us per group.
            pend = None
            for (jj, prs, closedj) in backlog:
                gs = epi_backlog(jj, prs, closedj)
                if pend is not None:
                    finish_xg(*pend)
                pend = (jj, gs)
            finish_xg(*pend)
            backlog.clear()

            # ---- bidirectional 1-step LSTM (i, g, o only), with the fc-head
            # accumulation matmuls interleaved one group behind the feat
            # writes (hp holds its own PSUM bank for the whole phase) ----
            hp = hpp.tile([128, BW], dt.float32, tag="hp", name="hp")

            def emit_head(k, plast):
                for b in range(NB):
                    mm = nc.tensor.matmul(
                        hp[b * 64:(b + 1) * 64, :], w1_sb[:, k * 64:(k + 1) * 64],
                        feat[:, k * BL + b * BW: k * BL + (b + 1) * BW],
                        start=(k == 0), stop=(k == KC - 1),
                    )
                    if b == 0 and plast is not None:
                        add_dep_helper(mm.ins, plast.ins,
                                       reason="head rides the lstm stream")
                return mm

            for d in range(2):
                for hm in range(8):
                    dh = d * 8 + hm
                    if dh < 4:
                        wlt = wl_tiles[dh]
                    else:
                        # one DMA per (d,hm): all three i/g/o part blocks
                        # (0.75MB), WAR-paced by the 4-deep lpool
                        wlt = lpool.tile([128, 3, KC, 128], dt.float8e4,
                                         tag="wl", name=f"wl{dh}")
                        nc.sync.dma_start(wlt[:], wl_d[dh])
                    pp = [
                        [psum.tile([128, BW], dt.float32, tag="mm",
                                   name=f"lp{dh * 3 + part}_{b}")
                         for b in range(NB)]
                        for part in range(3)
                    ]
                    # dh0 runs sc-outer: the trailing gate epilogues are
                    # still producing the last xg chunks when the handoff
                    # happens, so consume pairs in production order. Later
                    # groups run part-outer (2-bank pipelining).
                    if dh == 0:
                        order = [(sc, part) for sc in range(SC)
                                 for part in range(3)]
                    else:
                        order = [(sc, part) for part in range(3)
                                 for sc in range(SC)]
                    for (sc, part) in order:
                        for b in range(NB):
                            if sc == 0 and b == 0 and prev_last is not None:
                                chain_to = prev_last
                            else:
                                chain_to = None
                            mm = nc.tensor.matmul(
                                pp[part][b][:], wlt[:, part, 2 * sc:2 * sc + 2, :],
                                xg3[:, 2 * sc:2 * sc + 2, b * BW:(b + 1) * BW],
                                start=(sc == 0), stop=(sc == SC - 1),
                                perf_mode=DR,
                            )
                            if chain_to is not None:
                                add_dep_helper(mm.ins, chain_to.ins,
                                               reason="lstm groups in order")
                            prev_last = mm
                    bcol = d * 24 + hm
                    ti = tmp.tile([128, BL], dt.bfloat16, tag="ti", name=f"ti{dh}")
                    tg = tmp.tile([128, BL], dt.bfloat16, tag="tg", name=f"tg{dh}")
                    to = tmp.tile([128, BL], dt.bfloat16, tag="to", name=f"to{dh}")
                    for b in range(NB):
                        bs = slice(b * BW, (b + 1) * BW)
                        nc.scalar.activation(ti[:, bs], pp[0][b][:], AF.Sigmoid,
                                             bias=bi_sb[:, bcol:bcol + 1], scale=1.0 / WS)
                        nc.scalar.activation(tg[:, bs], pp[1][b][:], AF.Tanh,
                                             bias=bi_sb[:, bcol + 8:bcol + 9], scale=1.0 / WS)
                        nc.scalar.activation(to[:, bs], pp[2][b][:], AF.Sigmoid,
                                             bias=bi_sb[:, bcol + 16:bcol + 17], scale=1.0 / WS)
                    cb, tc2 = ti, tg  # in-place: c overwrites ti, tanh(c) tg
                    if dh < 15:
                        nc.vector.tensor_mul(cb[:], ti[:], tg[:])
                        nc.scalar.activation(tc2[:], cb[:], AF.Tanh)
                        nc.vector.tensor_mul(feat[:, dh * BL:(dh + 1) * BL],
                                             to[:], tc2[:])
                        if dh >= 1:
                            # head chunk dh-1: its feat was written while
                            # this group's matmuls ran, so it never stalls
                            prev_last = emit_head(dh - 1, prev_last)
                    else:
                        # last group: run the whole epilogue -> head -> relu
                        # -> W2 -> out chain per batch half so the serial
                        # tail pipelines across DVE/ACT/PE
                        prev_last = emit_head(dh - 1, prev_last)
                        for b in range(NB):
                            bs = slice(b * BW, (b + 1) * BW)
                            fs = slice(dh * BL + b * BW, dh * BL + (b + 1) * BW)
                            hs = slice(b * 64, (b + 1) * 64)
                            nc.vector.tensor_mul(cb[:, bs], ti[:, bs], tg[:, bs])
                            nc.scalar.activation(tc2[:, bs], cb[:, bs], AF.Tanh)
                            nc.vector.tensor_mul(feat[:, fs], to[:, bs], tc2[:, bs])
                            mm = nc.tensor.matmul(
                                hp[hs, :], w1_sb[:, (KC - 1) * 64:KC * 64],
                                feat[:, fs], start=False, stop=True)
                            add_dep_helper(mm.ins, prev_last.ins,
                                           reason="head tail per half")
                            prev_last = mm
                            nc.scalar.activation(hid2[hs, :], hp[hs, :],
                                                 AF.Relu, bias=b1_sb[hs, :])
                            op_ = psum.tile([1, BW], dt.float32, tag="mm",
                                            name=f"op{b}")
                            nc.tensor.matmul(op_[:], w2_sb[hs, :], hid2[hs, :])
                            nc.vector.tensor_scalar_add(
                                orow[:, b * BW:(b + 1) * BW], op_[:], b2_sb[:])
                            nc.sync.dma_start(out_d[:, b * BW:(b + 1) * BW],
                                              orow[:, b * BW:(b + 1) * BW])

    nc.compile()
    return nc


def _prep_inputs(x, ln_g, ln_b, Wg, bg, W_ih_f, b_ih_f, b_hh_f, W_ih_b, b_ih_b, b_hh_b,
                 W1, b1, W2, b2):
    """Host-side resharding/packing. All layouts are [partition, free]-grouped so
    every DMA lands as >=1KB contiguous runs per partition. Matmul weights are
    quantized to fp8-e4m3 after a x64 pre-scale."""
    f64 = np.float64

    def kgroup8(lhsT, mwidth):
        # lhsT [F, M] fp64 -> [M//mwidth groups][128 part][KC][mwidth] f8
        M = lhsT.shape[1]
        a = (lhsT * WS).astype(F8)
        a = a.reshape(KC, 128, M // mwidth, mwidth).transpose(2, 1, 0, 3)
        return np.ascontiguousarray(a)

    def kgroup(lhsT, mwidth):
        M = lhsT.shape[1]
        a = lhsT.reshape(KC, 128, M // mwidth, mwidth).transpose(2, 1, 0, 3)
        return np.ascontiguousarray(a.reshape(M // mwidth, 128, KC * mwidth)).astype(BF16)

    Wgl = (Wg.astype(f64) * ln_g.astype(f64)[None, :])
    wgm = kgroup8(np.ascontiguousarray(Wgl.T), 128)            # [16,128,16,128] f8
    # c1 from the quantized weights so the mu-correction matches the matmul
    Wq = wgm.astype(f64)                                       # [16,128,16,128]
    c1 = Wq.sum(axis=(1, 2)).reshape(16 * 128)                 # [2048] (x64 scale)
    # xe row 1 is exp(0.5*ln(WS^2*(var+eps))) = WS*sqrt(ve): c2 stays unscaled
    c2 = Wg.astype(f64) @ ln_b.astype(f64)                     # [2048]
    wge = np.stack([c1.reshape(16, 128), c2.reshape(16, 128)], axis=1).astype(BF16)

    idx = np.r_[0:H, 2 * H:3 * H, 3 * H:4 * H]                 # i, g, o rows
    wl_groups = []
    bl_all = np.zeros((128, 48), np.float32)
    for d, (Wih, bih, bhh) in enumerate(
        [(W_ih_f, b_ih_f, b_hh_f), (W_ih_b, b_ih_b, b_hh_b)]
    ):
        P = Wih[idx, :].astype(f64)                            # [3072, 2048]
        g24 = kgroup8(np.ascontiguousarray(P.T), 128)          # [24,128,16,128]
        for hm in range(8):
            # one [128, 3(part), KC, 128] block per (d,hm) -> single DMA
            wl_groups.append(np.stack([g24[part * 8 + hm] for part in range(3)], axis=1))
        bp = (bih.astype(f64) + bhh.astype(f64))[idx].astype(np.float32)
        bl_all[:, d * 24:(d + 1) * 24] = bp.reshape(24, 128).T  # col c = chunk p*8+hm
    wlm = np.ascontiguousarray(np.stack(wl_groups))            # [16,128,3,16,128]

    w1m = kgroup(np.ascontiguousarray(W1.T), 64)[0][None]      # [1,128,1024] -> squeeze
    w1m = np.ascontiguousarray(w1m[0])                         # [128, 16*64]
    w2m = np.ascontiguousarray(np.tile(W2[0], 2)[:, None]).astype(BF16)  # [128,1]
    bgm = np.ascontiguousarray(bg.reshape(16, 128).T).astype(np.float32)  # [128,16]

    shared = {
        "wgm": wgm, "wge": wge, "wlm": wlm, "blv": bl_all, "bgv": bgm,
        "c1v": np.ascontiguousarray(c1.reshape(16, 128).T).astype(np.float32),
        "w1v": w1m, "w2v": w2m,
        "b1v": np.ascontiguousarray(np.tile(np.asarray(b1), 2)[:, None]).astype(np.float32),
        "b2v": np.asarray(b2, np.float32).reshape(1, 1),
    }
    in_maps = []
    for c in range(NCORES):
        xs = x[c * BL:(c + 1) * BL, :].T                       # [2048, 1024]
        xt = np.ascontiguousarray(
            xs.reshape(KC, 128, BL).transpose(1, 0, 2)
        )                                                      # [128,16,1024] f32
        in_maps.append({"xt": xt.astype(BF16), "xq": xt.astype(F8),
                        "xsq": (xt * xt).astype(F8), **shared})
    return in_maps


def _run(in_maps, trace=False, has_c2=False):
    key = ("nc", has_c2)
    if key not in _CACHE:
        _CACHE[key] = _build_graph(has_c2=has_c2)
    res = bass_utils.run_bass_kernel_spmd(
        _CACHE[key], in_maps, core_ids=list(range(NCORES)), trace=trace
    )
    return res


def kernel(x, ln_g, ln_b, Wg, bg,
           W_ih_f, W_hh_f, b_ih_f, b_hh_f,
           W_ih_b, W_hh_b, b_ih_b, b_hh_b,
           W1, b1, W2, b2, _trace=False, _return_res=False):
    args = [np.asarray(a) for a in (x, ln_g, ln_b, Wg, bg, W_ih_f, b_ih_f, b_hh_f,
                                    W_ih_b, b_ih_b, b_hh_b, W1, b1, W2, b2)]
    in_maps = _prep_inputs(*args)
    has_c2 = bool(np.any(np.asarray(ln_b) != 0))
    res = _run(in_maps, trace=_trace, has_c2=has_c2)
    out = np.concatenate(
        [np.asarray(res.results[c]["out"]).reshape(-1) for c in range(NCORES)]
    ).astype(np.float32)
    if _return_res:
        return out, res
    return out



# revision 49
# speedup vs baseline: 1.0254x; 1.0254x over previous
"""Trainium2 Bass kernel for nn_AFSLSTM (LayerNorm -> sigmoid feature gate ->
bidirectional 1-step LSTM -> tiny MLP head).

Sharding: data-parallel over the batch dim, 1024 rows per core, weights
replicated. No collectives; host concatenates per-core outputs.

Device-side math (per core, feature-major layout [feature_part, batch_free]):
the gate and LSTM matmuls (97% of PE work) run as fp8-e4m3 DoubleRow matmuls
(two 128-row contraction chunks per instruction, ~213ns/MM spacing covering
256xK at N=512), with weights pre-scaled by 64 on the host so the
uniform(-1/32,1/32) entries sit in fp8's normal range. The 1/64 descale
folds into the LN rsqrt (gate) and the activation `scale` operand (LSTM).
x ships twice (fp8 for the gate moving operand + LN stats, bf16 for the xg
product); xg is written as fp8 by the DVE so the LSTM matmuls consume it
directly. Head stays bf16.
  G0 = Q8(64*Wg*ln_g).Q8(x)                  (8 DoubleRow matmuls per j-chunk)
  gate = sigmoid((rsq/64)*(G0 - mu(x)c1q) + bg)    (c1q = colsums of the
        quantized weights; ALL groups park their PSUM to SBUF via ScalarE
        copies right at close -- parking decouples bank recycling from the
        DVE epilogue drain, which starts only after the rsq broadcast and
        would otherwise WAR-stall the first LSTM group ~4.7us on the tail
        banks -- and the j >= CLOSE_FROM=12 tail groups apply -mu*c1 as a
        K=1 bf16 close matmul BEFORE parking, so their drain flush is the
        cheap a_b-scale path instead of the scalar_tensor_tensor one)
  xg = fp8(x * gate)
  pre_{i,g,o} = Q8(64*W_ih[{i,g,o}]).xg      (f-gate dropped: c0 = 0)
  h = sigmoid(pre_o/64 + b_o) * tanh(sigmoid(pre_i/64 + b_i)*tanh(pre_g/64 + b_g))
  hid = relu(W1.feat + b1);  out = W2.hid + b2
LN statistics: BOTH batch sums (from fp8 x) and square sums (from host-shipped
fp8 x^2 pair-chunks) come from DoubleRow ones-matmuls interleaved into gate
j0's DMA-paced stream; mu -> -mu -> mu_b broadcast lands ~30us, the variance
chain (merged WS^2 scaling, mu^2 on ACT Square in parallel, Ln/Exp with
prefetched tables) broadcasts rsq ~42us; the post-loop drain then flushes all
16 parked groups in chunk order with each group's xg mul emitted one group
behind its sigmoid so the DVE FIFO never head-of-line blocks on ACT.
Schedule notes (trace-derived): engine preambles end ~5-7us; HAM warmup MMs
run off a GPSIMD-memset tile from ~5.5us; first gate MM ~13us paced by wg0 +
fp8-x pair arrival (DMA rings cost ~33ns/descriptor: 128-descriptor
transfers floor at ~4.4us); 4 LSTM weight groups prefetch on the sync queue
behind the bf16-x tail so the gate->LSTM handoff never starves; the fc-head
accumulation matmuls ride one group behind the LSTM feat writes into a
dedicated PSUM bank; the last LSTM group's epilogue -> head -> W2 -> out
chain runs per batch-half to pipeline the tail. Engine laws learned the hard
way: GPSIMD elementwise offload backfires (shared SBUF port pair with DVE);
sync and scalar share 8 one-transfer-at-a-time DMA rings and a busy-ring
trigger blocks its ENGINE queue, so bulk triggers never ride ACT; the tile
scheduler reorders per-engine programs, so critical cross-engine orderings
need explicit add_dep edges; single-partition DVE row ops run ~2.5ns/elem.
Measured: 274.2us baseline -> 266.9-268.0us (rel err 7.4e-3, gate 2e-2);
remaining: ~221us DoubleRow matmul floor, ~7us preamble, ~3us teardown,
~2.5us wave-1 ring pacing, ~2us drain/gate balance at the LSTM handoff.
"""

import numpy as np
import ml_dtypes

import concourse.bacc as bacc
import concourse.bass as bass
import concourse.mybir as mybir
import concourse.tile as tile
from concourse import bass_utils

BF16 = ml_dtypes.bfloat16
F8 = ml_dtypes.float8_e4m3  # TRN FP8_EXP4 (max finite 240)
NCORES = 8
B, F, H = 8192, 2048, 1024
BL = B // NCORES          # 1024 rows per core
KC = F // 128             # 16 contraction chunks
SC = KC // 2              # 8 DoubleRow super-chunks
NB = 2                    # batch sub-chunks per core
BW = BL // NB             # 512 (one PSUM bank of fp32)
NG_LSTM = 2 * 8 * 3       # dir x h-chunk x {i,f,g,o}->{i,g,o} weight groups
EPS = 1e-5
WS = 64.0                 # fp8 weight pre-scale
USE_CLOSE_MM = True       # rank-1 PE close matmul for the -mu*c1 term
NBL = 16                  # ALL gate groups park to SBUF via ACT copies right
                          # at close: the j12-15 banks otherwise wait for the
                          # end of the DVE drain and WAR-stall the first LSTM
                          # group ~4.7us. The j >= CLOSE_FROM groups get the
                          # rank-1 close matmul BEFORE parking, so their praw
                          # already holds -mu*c1 and their flush is the cheap
                          # a_b-scale path (the all-stt NBL=16 variant pushed
                          # the xg tail past the LSTM start and lost ~2us).
CLOSE_FROM = 8            # 8 close-matmul'd groups balance the PE stream
                          # (+0.76us each) against the DVE drain (-0.81us
                          # each): both end ~84us instead of PE 80.6 / DVE 87.6

_CACHE = {}


def _build_graph(has_c2=False):
    """has_c2: general path with a nonzero ln_b (rank-1 close matmuls on PE).
    The fast path (ln_b == 0) applies the -mu*c1 correction as a fused DVE
    scalar_tensor_tensor against a GPSIMD partition-broadcast of -mu, so PSUM
    groups close right at the last K-chunk and PE never waits on LN stats."""
    dt = mybir.dt
    AF = mybir.ActivationFunctionType
    OP = mybir.AluOpType
    DR = mybir.MatmulPerfMode.DoubleRow

    nc = bacc.Bacc("TRN2", target_bir_lowering=False, debug=False)

    xt_d = nc.dram_tensor("xt", (128, KC, BL), dt.bfloat16, kind="ExternalInput")
    xq_d = nc.dram_tensor("xq", (128, KC, BL), dt.float8e4, kind="ExternalInput")
    xs_d = nc.dram_tensor("xsq", (128, KC, BL), dt.float8e4, kind="ExternalInput")
    wg_d = nc.dram_tensor("wgm", (16, 128, KC, 128), dt.float8e4, kind="ExternalInput")
    wge_d = nc.dram_tensor("wge", (16, 2, 128), dt.bfloat16, kind="ExternalInput")
    wl_d = nc.dram_tensor("wlm", (16, 128, 3, KC, 128), dt.float8e4, kind="ExternalInput")
    bg_d = nc.dram_tensor("bgv", (128, 16), dt.float32, kind="ExternalInput")
    c1_d = nc.dram_tensor("c1v", (128, 16), dt.float32, kind="ExternalInput")
    bl_d = nc.dram_tensor("blv", (128, 48), dt.float32, kind="ExternalInput")
    w1_d = nc.dram_tensor("w1v", (128, KC * 64), dt.bfloat16, kind="ExternalInput")
    w2_d = nc.dram_tensor("w2v", (128, 1), dt.bfloat16, kind="ExternalInput")
    b1_d = nc.dram_tensor("b1v", (128, 1), dt.float32, kind="ExternalInput")
    b2_d = nc.dram_tensor("b2v", (1, 1), dt.float32, kind="ExternalInput")
    out_d = nc.dram_tensor("out", (1, BL), dt.float32, kind="ExternalOutput")

    with tile.TileContext(nc) as tc:
        with (
            tc.tile_pool(name="pers", bufs=1) as pers,
            tc.tile_pool(name="wpool", bufs=6) as wpool,
            tc.tile_pool(name="lpool", bufs=4) as lpool,
            tc.tile_pool(name="wepool", bufs=3) as wepool,
            tc.tile_pool(name="sqp", bufs=7) as sqp,
            tc.tile_pool(name="tmp", bufs=2) as tmp,
            tc.tile_pool(name="psum", bufs=7, space=bass.MemorySpace.PSUM) as psum,
            tc.tile_pool(name="hpp", bufs=1, space=bass.MemorySpace.PSUM) as hpp,
        ):
            # ---- persistent SBUF tensors ----
            xsb = pers.tile([128, KC * BL], dt.bfloat16)
            xq3 = pers.tile([128, KC, BL], dt.float8e4)
            xg3 = pers.tile([128, KC, BL], dt.float8e4)
            feat = pers.tile([128, KC * BL], dt.bfloat16)
            hid2 = pers.tile([128, BW], dt.bfloat16)
            a_b = pers.tile([128, BL], dt.bfloat16)      # rsq/WS broadcast tile
            mu_b = pers.tile([128, BL], dt.bfloat16)     # -mu broadcast tile
            ones128 = pers.tile([128, 1], dt.bfloat16)
            onesq = pers.tile([128, 2, 16], dt.float8e4)  # DR ones lhsT (col 0 used)
            wtile = pers.tile([128, 128], dt.bfloat16)   # HAM warmup operand
            # single-partition f32 rows (each costs free-bytes on every
            # partition, so reuse aggressively): s1row doubles as mu,
            # s2row as t1 -> ve -> lnv, orow doubles as the var rowtmp.
            s1row = pers.tile([1, BL], dt.float32)
            s2row = pers.tile([1, BL], dt.float32)
            rsqb = pers.tile([1, BL], dt.bfloat16)
            orow = pers.tile([1, BL], dt.float32)
            xe = pers.tile([2, BL], dt.bfloat16)         # rank-1 rhs rows (-mu, sqrt(ve))
            xe1s = (pers.tile([1, BL], dt.bfloat16)      # partition-0 staging, row 1
                    if has_c2 else None)
            bg_sb = pers.tile([128, 16], dt.float32)
            c1_sb = pers.tile([128, 16], dt.float32)
            bi_sb = pers.tile([128, 48], dt.float32)     # col = d*24 + part*8 + hm
            w1_sb = pers.tile([128, KC * 64], dt.bfloat16)
            w2_sb = pers.tile([128, 1], dt.bfloat16)
            b1_sb = pers.tile([128, 1], dt.float32)
            b2_sb = pers.tile([1, 1], dt.float32)

            nc.vector.memset(ones128[:], 1.0)
            nc.vector.memset(onesq[:], 1.0)
            nc.vector.memset(a_b[:], 0.0)
            # dummy matmuls on a zeroed tile keep the PE busy through the
            # wave-1 DMA wait so the HAM clock gate is at 8/8 (2.4GHz) when
            # the real gate stream starts. The operand is memset on GPSIMD,
            # whose ~5us engine preamble finishes ~2us before Vector's, so
            # the warmups begin right as the Tensor queue opens.
            nc.gpsimd.memset(wtile[:], 0.0)
            warmp = psum.tile([128, 128], dt.float32, tag="mm", name="warmp")
            for _ in range(60):
                nc.tensor.matmul(warmp[:], wtile[:], wtile[:],
                                 start=True, stop=True)

            # ---- input schedule. The sync and scalar engines share 8 DMA
            # rings; each ring runs ONE transfer at a time (~43GB/s) and a
            # trigger whose ring is busy blocks its ENGINE queue (so bulk
            # triggers must never ride a compute engine: the scalar/ACT
            # queue only gets the small wave-1 odd chunks, issued before any
            # ACT work exists). Deadline order on sync:
            #   wg0 (4 subs, first matmul ~9.5us) -> xq/xsq wave 1 (4.7MB,
            #   j0's DMA-paced stream consumes it as it lands) -> wg1..3 ->
            #   wg4..15 (chained 3-deep so they complete in j order) ->
            #   consts -> wets -> bf16 x (first gate epilogue needs xt0
            #   ~35us) -> wl0..3 LSTM prefetch -> in-loop wl4..15 (WAR-paced
            #   by lpool slots).
            # NOTE: never chain DMAs across streams (an xt<-wg edge deadlocks:
            # wg slots free only after gate MMs that sit behind the stats MMs
            # on the in-order PE queue, and stats need xt).
            from concourse.tile_rust import add_dep_helper
            wgt01 = [wpool.tile([128, KC, 128], dt.float8e4, tag="w", name=f"wg{j}")
                     for j in range(2)]

            for s in range(2):
                nc.sync.dma_start(
                    wgt01[0][:, s * 8:(s + 1) * 8, :],
                    wg_d[0, :, s * 8:(s + 1) * 8, :],
                )
            # wave 1: fp8 x and x^2 ship as PAIR-chunk transfers (2KB
            # contiguous runs per partition -- the ~33ns/descriptor ring
            # overhead dominates smaller runs), spread across both trigger
            # engines so all 8 rings fill in parallel
            xqd = [None] * SC
            xsqt = [None] * SC
            xsqd = [None] * SC

            def xsq_pair(sc, eng):
                t = sqp.tile([128, 2, BL], dt.float8e4, tag="xsq2", name=f"xsq{sc}")
                xsqt[sc] = t
                xsqd[sc] = eng.dma_start(t[:], xs_d[:, 2 * sc:2 * sc + 2, :])

            nc.sync.dma_start(xq3[:, 0, :], xq_d[:, 0, :])
            xqd[0] = nc.scalar.dma_start(xq3[:, 1, :], xq_d[:, 1, :])
            for sc in range(1, SC):
                eng = nc.sync if sc % 2 == 0 else nc.scalar
                xqd[sc] = eng.dma_start(xq3[:, 2 * sc:2 * sc + 2, :],
                                        xq_d[:, 2 * sc:2 * sc + 2, :])
            for sc in range(SC):
                xsq_pair(sc, nc.sync if sc % 2 == 1 else nc.scalar)
            xqdma = xqd[7]

            wgts = {0: wgt01[0], 1: wgt01[1]}
            wgd = {}
            for s in range(2):
                wgd[1] = nc.sync.dma_start(
                    wgt01[1][:, s * 8:(s + 1) * 8, :],
                    wg_d[1, :, s * 8:(s + 1) * 8, :],
                )
                add_dep_helper(wgd[1].ins, xqdma.ins, reason="wg1 behind wave 1")
            for j in range(2, 16):
                wgt = wpool.tile([128, KC, 128], dt.float8e4, tag="w", name=f"wg{j}")
                wd = nc.sync.dma_start(wgt[:], wg_d[j])
                # chain 3-deep so the weights complete in j order instead of
                # fair-sharing the rings and all landing together at the end
                dep = wgd[j - 3] if j - 3 >= 1 else None
                add_dep_helper(wd.ins, (dep or xqdma).ins,
                               reason="gate weight chain")
                wgd[j] = wd
                wgts[j] = wgt

            # ---- constants / small DMAs (held behind the gate weights) ----
            for sb_t, dr_t in [(bg_sb, bg_d), (c1_sb, c1_d), (bi_sb, bl_d),
                               (w1_sb, w1_d), (w2_sb, w2_d), (b1_sb, b1_d),
                               (b2_sb, b2_d)]:
                cd = nc.sync.dma_start(sb_t[:], dr_t[:, :])
                add_dep_helper(cd.ins, xqdma.ins, reason="const dma behind wave 1")

            # ---- LN statistics, entirely off the DVE/ACT hot paths: both
            # the batch sums s1 (from fp8 x) and the square sums s2 (from the
            # host-shipped fp8 x^2 pair-chunks) come from DoubleRow
            # ones-matmuls interleaved into gate j0's DMA-paced stream, so
            # they fill the PE idle slivers while wave 1 lands and the full
            # stats chain + broadcasts retire by ~34us (mu's fp8 quantization
            # error is ~6e-4 absolute, x^2's quantization adds ~0.2% to var
            # -- negligible vs the fp8 matmul noise).
            def emit_stat_mm(dst, rhs_pair, first, will_stop):
                for b in range(NB):
                    mm = nc.tensor.matmul(
                        dst[b][:], onesq[:, :, 0:1],
                        rhs_pair[:, :, b * BW:(b + 1) * BW],
                        start=first, stop=will_stop, perf_mode=DR,
                    )
                return mm

            def emit_mu():
                # evacuate s1, fold to mu = s1/F in place, and write the
                # -mu rank-1 row the (tail-group) close matmuls consume.
                for b in range(NB):
                    nc.vector.tensor_copy(s1row[:, b * BW:(b + 1) * BW], s1p[b][:])
                nc.vector.tensor_scalar_mul(s1row[:], s1row[:], 1.0 / F)
                nc.vector.tensor_scalar_mul(xe[0:1, :], s1row[:], -1.0)  # -> bf16
                nc.gpsimd.partition_broadcast(mu_b[:], xe[0:1, :])

            def emit_stats_chain():
                # variance chain from the s2 PSUM partials, then the
                # -mu / rsq partition-broadcasts on the idle GPSIMD.
                mu, lnv = s1row, s2row  # s1row already holds mu
                for b in range(NB):
                    nc.vector.tensor_copy(s2row[:, b * BW:(b + 1) * BW], s2p[b][:])
                # WS^2-prescaled var chain (exp(-0.5 ln(.)) then yields
                # rsq/WS, undoing the fp8 weight pre-scale). Single-partition
                # DVE ops run ~2.5ns/elem, so the mu^2 term goes to ACT
                # (Square with scale=WS), overlapping the DVE scale op.
                nc.vector.tensor_scalar(s2row[:], s2row[:], WS * WS / F,
                                        WS * WS * EPS, OP.mult, OP.add)
                sq = nc.scalar.activation(orow[:], mu[:], AF.Square, scale=WS)
                sq_ref.append(sq)
                nc.vector.tensor_sub(s2row[:], s2row[:], orow[:])  # WS^2(var+eps)
                nc.scalar.activation(lnv[:], s2row[:], AF.Ln)
                nc.scalar.activation(rsqb[:], lnv[:], AF.Exp, scale=-0.5)
                nc.gpsimd.partition_broadcast(a_b[:], rsqb[:])
                if has_c2:
                    # engines can only write partition bases {0,32,64,96};
                    # stage row 1 on partition 0 and DMA it into place.
                    nc.scalar.activation(xe1s[:], lnv[:], AF.Exp, scale=0.5)
                    nc.sync.dma_start(xe[1:2, :], xe1s[:])

            def finish_gate(j, tps):
                gs = tmp.tile([128, BL], dt.bfloat16, tag="gs", bufs=3, name=f"gs{j}")
                nc.scalar.activation(gs[:], tps[:], AF.Sigmoid, bias=bg_sb[:, j:j + 1])
                return gs

            def finish_xg(j, gs):
                nc.vector.tensor_mul(xg3[:, j, :], xsb[:, j * BL:(j + 1) * BL], gs[:])

            def epi_backlog(j, prs, closed=False):
                # deferred path (stats weren't ready when the group closed):
                # praw was parked in SBUF; apply the -mu*c1 correction with a
                # DVE scalar_tensor_tensor, then scale, sigmoid, gate.
                tpre = tmp.tile([128, BL], dt.bfloat16, tag="tpre", bufs=3, name=f"tp{j}")
                for b in range(NB):
                    if has_c2 or closed:
                        nc.vector.tensor_mul(tpre[:, b * BW:(b + 1) * BW], prs[b][:],
                                             a_b[:, b * BW:(b + 1) * BW])
                    else:
                        nc.vector.scalar_tensor_tensor(
                            tpre[:, b * BW:(b + 1) * BW], mu_b[:, b * BW:(b + 1) * BW],
                            c1_sb[:, j:j + 1], prs[b][:], OP.mult, OP.add,
                        )
                if not (has_c2 or closed):
                    nc.vector.tensor_mul(tpre[:], tpre[:], a_b[:])
                return finish_gate(j, tpre)

            def epi_live(j, gp, closed):
                # live path: the rank-1 close matmul already applied -mu*c1 in
                # PSUM, so one DVE mul per bank both scales by rsq/WS and
                # evacuates (frees) the bank.
                tps = tmp.tile([128, BL], dt.bfloat16, tag="tps", name=f"ts{j}")
                for b in range(NB):
                    if closed:
                        nc.vector.tensor_mul(tps[:, b * BW:(b + 1) * BW], gp[b][:],
                                             a_b[:, b * BW:(b + 1) * BW])
                    else:
                        nc.vector.scalar_tensor_tensor(
                            tps[:, b * BW:(b + 1) * BW], mu_b[:, b * BW:(b + 1) * BW],
                            c1_sb[:, j:j + 1], gp[b][:], OP.mult, OP.add,
                        )
                if not closed:
                    nc.vector.tensor_mul(tps[:], tps[:], a_b[:])
                return finish_gate(j, tps)

            # ---- feature gate: 16 j-chunks x 8 DoubleRow super-chunks. PSUM
            # groups are evacuated to SBUF (bf16) right as they close; the
            # epilogues of j0..j3 are deferred until after the stats chain is
            # emitted so the DVE stream never blocks on the LN broadcasts.
            def gate_mm(gp, wgt, j, sc, will_close):
                out = []
                for b in range(NB):
                    out.append(nc.tensor.matmul(
                        gp[b][:], wgt[:, 2 * sc:2 * sc + 2, :],
                        xq3[:, 2 * sc:2 * sc + 2, b * BW:(b + 1) * BW],
                        start=(sc == 0), stop=(not will_close and sc == SC - 1),
                        perf_mode=DR,
                    ))
                return out

            def gate_close_evac(j, wet, gp, backlog):
                closed = (j >= CLOSE_FROM and USE_CLOSE_MM) or has_c2
                if (backlog is None and closed):
                    # rank-1 close matmul: += (-mu)(x)c1 (and sqrt(ve)(x)c2 in
                    # the general path) applied in PSUM at ~0.2us/bank of PE,
                    # saving the DVE scalar_tensor_tensor on the live path
                    K2 = 2 if has_c2 else 1
                    for b in range(NB):
                        nc.tensor.matmul(
                            gp[b][:], wet[0:K2, :], xe[0:K2, b * BW:(b + 1) * BW],
                            start=False, stop=True,
                        )
                prs = []
                for b in range(NB):
                    praw = tmp.tile([128, BW], dt.bfloat16, tag="praw", bufs=8,
                                    name=f"pr{j}_{b}")
                    # ScalarE copy: prompt bank release off the loaded DVE
                    cp = nc.scalar.copy(praw[:], gp[b][:])
                    if j == 1 and sq_ref:
                        add_dep_helper(cp.ins, sq_ref[0].ins,
                                       reason="square ahead of j1 parks")
                    prs.append(praw)
                backlog.append((j, prs, closed))

            wets = {}
            for j in range(16):
                if has_c2 or j >= CLOSE_FROM:
                    wet = wepool.tile([2, 128], dt.bfloat16, tag="we", bufs=6,
                                      name=f"we{j}")
                    wed = nc.sync.dma_start(wet[:], wge_d[j, :, :])
                    add_dep_helper(wed.ins, wgd[12].ins,
                                   reason="rank-1 rows late")
                    wets[j] = wet

            # bf16 x: consumed only by the gate epilogues (xg = x*gate), the
            # first of which waits for the stats broadcasts ~46us; stream it
            # behind the gate weights and consts on the sync queue.
            xtd = []
            for q in range(KC):
                xd = nc.sync.dma_start(xsb[:, q * BL:(q + 1) * BL], xt_d[:, q, :])
                add_dep_helper(xd.ins, wgd[min(15, 4 + q)].ins,
                               reason="bf16 x staggered behind weights")
                xtd.append(xd)

            # LSTM weight prefetch: the first 4 groups ride the sync queue
            # behind the bf16 x tail so the gate->LSTM handoff never starves
            # (they land ~50us, needed ~70); the rest stream in-loop,
            # WAR-paced by lpool slots. NEVER on the scalar engine: a
            # ring-busy trigger there blocks the whole ACT queue.
            wl_tiles = {}
            for dh in range(4):
                wlt = lpool.tile([128, 3, KC, 128], dt.float8e4, tag="wl",
                                 name=f"wl{dh}")
                wld = nc.sync.dma_start(wlt[:], wl_d[dh])
                add_dep_helper(wld.ins, xtd[12 + dh].ins,
                               reason="lstm prefetch behind bf16 x")
                wl_tiles[dh] = wlt

            # warm ONLY the Ln/Exp tables while ACT idles behind the wave-1
            # triggers: warming Sigmoid/Tanh too evicts these (the slots
            # cycle) and the real Ln/Exp reload on the rsq critical path
            for fn in (AF.Ln, AF.Exp):
                nc.scalar.activation(orow[0:1, 0:1], ones128[0:1, 0:1], fn)

            backlog = []
            live_defer = []
            sq_ref = []
            prev_last = None
            s1p = []
            s2p = []
            # Each j's first matmul is chained on the previous j's last: the
            # PE is serial anyway, but without the explicit edge the scheduler
            # interleaves the groups, closing them all late and starving PSUM
            # bank recycling (measured ~2-4us stall at j3/j4 on the bf16 ver).
            for j in range(16):
                wgt = wgts[j]
                wet = wets.get(j)
                gp = [psum.tile([128, BW], dt.float32, tag="mm", name=f"gp{j}_{b}")
                      for b in range(NB)]
                if j == 0:
                    s1p.extend(psum.tile([1, BW], dt.float32, tag="mm",
                                         name=f"s1p{b}") for b in range(NB))
                    s2p.extend(psum.tile([1, BW], dt.float32, tag="mm",
                                         name=f"s2p{b}") for b in range(NB))
                will_close = has_c2 or (j >= CLOSE_FROM and USE_CLOSE_MM)
                stat_last = None
                for sc in range(SC):
                    mms = gate_mm(gp, wgt, j, sc, will_close)
                    if sc == 0 and prev_last is not None:
                        add_dep_helper(mms[0].ins, prev_last.ins,
                                       reason="gate groups close in order")
                    if j == 0:
                        # the s1/s2 stat matmuls ride j0's DMA-paced
                        # chunk-pair stream, filling the PE idle slivers
                        # while wave 1 arrives (s2 lags 2 pairs so its
                        # fp8 x^2 transfer has landed)
                        stat_last = emit_stat_mm(
                            s1p, xq3[:, 2 * sc:2 * sc + 2, :],
                            sc == 0, sc == SC - 1)
                        if sc >= 2:
                            stat_last = emit_stat_mm(
                                s2p, xsqt[sc - 2][:], sc == 2, False)
                if j == 0:
                    for sc in (SC - 2, SC - 1):
                        stat_last = emit_stat_mm(s2p, xsqt[sc][:], False,
                                                 sc == SC - 1)
                prev_last = stat_last if j == 0 else mms[-1]
                gate_close_evac(j, wet, gp, backlog if j < NBL else None)
                if j == 0:
                    # stats resolve while j1/j2 stream: mu ~24us, broadcasts
                    # ~30-34us, so the j4+ epilogues run live
                    emit_mu()
                    emit_stats_chain()
            # post-loop DVE drain, in xg-production order: the 12 parked
            # flushes first (they only need mu_b/a_b, ~31us), then the tail
            # groups' PSUM evacuations. Each group's xg mul is emitted AFTER
            # the next group's stt/scale ops: the xg waits its sigmoid (ACT),
            # and emitting it inline would head-of-line block the DVE FIFO
            # for ~1.2us per group.
            pend = None
            for (jj, prs, closedj) in backlog:
                gs = epi_backlog(jj, prs, closedj)
                if pend is not None:
                    finish_xg(*pend)
                pend = (jj, gs)
            finish_xg(*pend)
            backlog.clear()

            # ---- bidirectional 1-step LSTM (i, g, o only), with the fc-head
            # accumulation matmuls interleaved one group behind the feat
            # writes (hp holds its own PSUM bank for the whole phase) ----
            hp = hpp.tile([128, BW], dt.float32, tag="hp", name="hp")

            def emit_head(k, plast):
                for b in range(NB):
                    mm = nc.tensor.matmul(
                        hp[b * 64:(b + 1) * 64, :], w1_sb[:, k * 64:(k + 1) * 64],
                        feat[:, k * BL + b * BW: k * BL + (b + 1) * BW],
                        start=(k == 0), stop=(k == KC - 1),
                    )
                    if b == 0 and plast is not None:
                        add_dep_helper(mm.ins, plast.ins,
                                       reason="head rides the lstm stream")
                return mm

            for d in range(2):
                for hm in range(8):
                    dh = d * 8 + hm
                    if dh < 4:
                        wlt = wl_tiles[dh]
                    else:
                        # one DMA per (d,hm): all three i/g/o part blocks
                        # (0.75MB), WAR-paced by the 4-deep lpool
                        wlt = lpool.tile([128, 3, KC, 128], dt.float8e4,
                                         tag="wl", name=f"wl{dh}")
                        nc.sync.dma_start(wlt[:], wl_d[dh])
                    pp = [
                        [psum.tile([128, BW], dt.float32, tag="mm",
                                   name=f"lp{dh * 3 + part}_{b}")
                         for b in range(NB)]
                        for part in range(3)
                    ]
                    # dh0 runs sc-outer: the trailing gate epilogues are
                    # still producing the last xg chunks when the handoff
                    # happens, so consume pairs in production order. Later
                    # groups run part-outer (2-bank pipelining).
                    if dh == 0:
                        order = [(sc, part) for sc in range(SC)
                                 for part in range(3)]
                    else:
                        order = [(sc, part) for part in range(3)
                                 for sc in range(SC)]
                    for (sc, part) in order:
                        for b in range(NB):
                            if sc == 0 and b == 0 and prev_last is not None:
                                chain_to = prev_last
                            else:
                                chain_to = None
                            mm = nc.tensor.matmul(
                                pp[part][b][:], wlt[:, part, 2 * sc:2 * sc + 2, :],
                                xg3[:, 2 * sc:2 * sc + 2, b * BW:(b + 1) * BW],
                                start=(sc == 0), stop=(sc == SC - 1),
                                perf_mode=DR,
                            )
                            if chain_to is not None:
                                add_dep_helper(mm.ins, chain_to.ins,
                                               reason="lstm groups in order")
                            prev_last = mm
                    bcol = d * 24 + hm
                    ti = tmp.tile([128, BL], dt.bfloat16, tag="ti", name=f"ti{dh}")
                    tg = tmp.tile([128, BL], dt.bfloat16, tag="tg", name=f"tg{dh}")
                    to = tmp.tile([128, BL], dt.bfloat16, tag="to", name=f"to{dh}")
                    for b in range(NB):
                        bs = slice(b * BW, (b + 1) * BW)
                        nc.scalar.activation(ti[:, bs], pp[0][b][:], AF.Sigmoid,
                                             bias=bi_sb[:, bcol:bcol + 1], scale=1.0 / WS)
                        nc.scalar.activation(tg[:, bs], pp[1][b][:], AF.Tanh,
                                             bias=bi_sb[:, bcol + 8:bcol + 9], scale=1.0 / WS)
                        nc.scalar.activation(to[:, bs], pp[2][b][:], AF.Sigmoid,
                                             bias=bi_sb[:, bcol + 16:bcol + 17], scale=1.0 / WS)
                    cb, tc2 = ti, tg  # in-place: c overwrites ti, tanh(c) tg
                    if dh < 15:
                        nc.vector.tensor_mul(cb[:], ti[:], tg[:])
                        nc.scalar.activation(tc2[:], cb[:], AF.Tanh)
                        nc.vector.tensor_mul(feat[:, dh * BL:(dh + 1) * BL],
                                             to[:], tc2[:])
                        if dh >= 1:
                            # head chunk dh-1: its feat was written while
                            # this group's matmuls ran, so it never stalls
                            prev_last = emit_head(dh - 1, prev_last)
                    else:
                        # last group: run the whole epilogue -> head -> relu
                        # -> W2 -> out chain per batch half so the serial
                        # tail pipelines across DVE/ACT/PE
                        prev_last = emit_head(dh - 1, prev_last)
                        for b in range(NB):
                            bs = slice(b * BW, (b + 1) * BW)
                            fs = slice(dh * BL + b * BW, dh * BL + (b + 1) * BW)
                            hs = slice(b * 64, (b + 1) * 64)
                            nc.vector.tensor_mul(cb[:, bs], ti[:, bs], tg[:, bs])
                            nc.scalar.activation(tc2[:, bs], cb[:, bs], AF.Tanh)
                            nc.vector.tensor_mul(feat[:, fs], to[:, bs], tc2[:, bs])
                            mm = nc.tensor.matmul(
                                hp[hs, :], w1_sb[:, (KC - 1) * 64:KC * 64],
                                feat[:, fs], start=False, stop=True)
                            add_dep_helper(mm.ins, prev_last.ins,
                                           reason="head tail per half")
                            prev_last = mm
                            nc.scalar.activation(hid2[hs, :], hp[hs, :],
                                                 AF.Relu, bias=b1_sb[hs, :])
                            op_ = psum.tile([1, BW], dt.float32, tag="mm",
                                            name=f"op{b}")
                            nc.tensor.matmul(op_[:], w2_sb[hs, :], hid2[hs, :])
                            nc.vector.tensor_scalar_add(
                                orow[:, b * BW:(b + 1) * BW], op_[:], b2_sb[:])
                            nc.sync.dma_start(out_d[:, b * BW:(b + 1) * BW],
                                              orow[:, b * BW:(b + 1) * BW])

    nc.compile()
    return nc


def _prep_inputs(x, ln_g, ln_b, Wg, bg, W_ih_f, b_ih_f, b_hh_f, W_ih_b, b_ih_b, b_hh_b,
                 W1, b1, W2, b2):
    """Host-side resharding/packing. All layouts are [partition, free]-grouped so
    every DMA lands as >=1KB contiguous runs per partition. Matmul weights are
    quantized to fp8-e4m3 after a x64 pre-scale."""
    f64 = np.float64

    def kgroup8(lhsT, mwidth):
        # lhsT [F, M] fp64 -> [M//mwidth groups][128 part][KC][mwidth] f8
        M = lhsT.shape[1]
        a = (lhsT * WS).astype(F8)
        a = a.reshape(KC, 128, M // mwidth, mwidth).transpose(2, 1, 0, 3)
        return np.ascontiguousarray(a)

    def kgroup(lhsT, mwidth):
        M = lhsT.shape[1]
        a = lhsT.reshape(KC, 128, M // mwidth, mwidth).transpose(2, 1, 0, 3)
        return np.ascontiguousarray(a.reshape(M // mwidth, 128, KC * mwidth)).astype(BF16)

    Wgl = (Wg.astype(f64) * ln_g.astype(f64)[None, :])
    wgm = kgroup8(np.ascontiguousarray(Wgl.T), 128)            # [16,128,16,128] f8
    # c1 from the quantized weights so the mu-correction matches the matmul
    Wq = wgm.astype(f64)                                       # [16,128,16,128]
    c1 = Wq.sum(axis=(1, 2)).reshape(16 * 128)                 # [2048] (x64 scale)
    # xe row 1 is exp(0.5*ln(WS^2*(var+eps))) = WS*sqrt(ve): c2 stays unscaled
    c2 = Wg.astype(f64) @ ln_b.astype(f64)                     # [2048]
    wge = np.stack([c1.reshape(16, 128), c2.reshape(16, 128)], axis=1).astype(BF16)

    idx = np.r_[0:H, 2 * H:3 * H, 3 * H:4 * H]                 # i, g, o rows
    wl_groups = []
    bl_all = np.zeros((128, 48), np.float32)
    for d, (Wih, bih, bhh) in enumerate(
        [(W_ih_f, b_ih_f, b_hh_f), (W_ih_b, b_ih_b, b_hh_b)]
    ):
        P = Wih[idx, :].astype(f64)                            # [3072, 2048]
        g24 = kgroup8(np.ascontiguousarray(P.T), 128)          # [24,128,16,128]
        for hm in range(8):
            # one [128, 3(part), KC, 128] block per (d,hm) -> single DMA
            wl_groups.append(np.stack([g24[part * 8 + hm] for part in range(3)], axis=1))
        bp = (bih.astype(f64) + bhh.astype(f64))[idx].astype(np.float32)
        bl_all[:, d * 24:(d + 1) * 24] = bp.reshape(24, 128).T  # col c = chunk p*8+hm
    wlm = np.ascontiguousarray(np.stack(wl_groups))            # [16,128,3,16,128]

    w1m = kgroup(np.ascontiguousarray(W1.T), 64)[0][None]      # [1,128,1024] -> squeeze
    w1m = np.ascontiguousarray(w1m[0])                         # [128, 16*64]
    w2m = np.ascontiguousarray(np.tile(W2[0], 2)[:, None]).astype(BF16)  # [128,1]
    bgm = np.ascontiguousarray(bg.reshape(16, 128).T).astype(np.float32)  # [128,16]

    shared = {
        "wgm": wgm, "wge": wge, "wlm": wlm, "blv": bl_all, "bgv": bgm,
        "c1v": np.ascontiguousarray(c1.reshape(16, 128).T).astype(np.float32),
        "w1v": w1m, "w2v": w2m,
        "b1v": np.ascontiguousarray(np.tile(np.asarray(b1), 2)[:, None]).astype(np.float32),
        "b2v": np.asarray(b2, np.float32).reshape(1, 1),
    }
    in_maps = []
    for c in range(NCORES):
        xs = x[c * BL:(c + 1) * BL, :].T                       # [2048, 1024]
        xt = np.ascontiguousarray(
            xs.reshape(KC, 128, BL).transpose(1, 0, 2)
        )                                                      # [128,16,1024] f32
        in_maps.append({"xt": xt.astype(BF16), "xq": xt.astype(F8),
                        "xsq": (xt * xt).astype(F8), **shared})
    return in_maps


def _run(in_maps, trace=False, has_c2=False):
    key = ("nc", has_c2)
    if key not in _CACHE:
        _CACHE[key] = _build_graph(has_c2=has_c2)
    res = bass_utils.run_bass_kernel_spmd(
        _CACHE[key], in_maps, core_ids=list(range(NCORES)), trace=trace
    )
    return res


def kernel(x, ln_g, ln_b, Wg, bg,
           W_ih_f, W_hh_f, b_ih_f, b_hh_f,
           W_ih_b, W_hh_b, b_ih_b, b_hh_b,
           W1, b1, W2, b2, _trace=False, _return_res=False):
    args = [np.asarray(a) for a in (x, ln_g, ln_b, Wg, bg, W_ih_f, b_ih_f, b_hh_f,
                                    W_ih_b, b_ih_b, b_hh_b, W1, b1, W2, b2)]
    in_maps = _prep_inputs(*args)
    has_c2 = bool(np.any(np.asarray(ln_b) != 0))
    res = _run(in_maps, trace=_trace, has_c2=has_c2)
    out = np.concatenate(
        [np.asarray(res.results[c]["out"]).reshape(-1) for c in range(NCORES)]
    ).astype(np.float32)
    if _return_res:
        return out, res
    return out

